# revision 2
# baseline (speedup 1.0000x reference)
r"""Boson-sampling probability |Perm(A)|^2 via Glynn's formula on 8 Trainium2 cores.

Math
----
perm(A) = 2^(1-n) * sum_{d in {-1,+1}^n} (prod_i d_i) * prod_j (sum_i d_i A_ij), n=20.
Terms for d and -d are equal, so enumerate d_19 = -1 only and double.

Sign-bit allocation for the remaining 19 bits:
  bits 0..8   -> free axis f (512)       [same on every core]
  bits 9..15  -> partition axis p (128)  [same on every core]
  bits 16..18 -> core c (8)

Row vector V_j(p,f,c) = Cp_c[p,j] + Cf[f,j] with
  Cp_c[p,j] = sum_{i=9..15} d_i(p) A[i,j] + sum_{i=16..18} d_i(c) A[i,j] - A[19,j]
  Cf[f,j]   = sum_{i=0..8} d_i(f) A[i,j]

Split the j-product into groups GA=0..6, GB=7..13, GC=14..19. Each group
product expands over subsets T of the group:
  PG[p,f] = sum_T (prod_{j in T} Cp[p,j]) * (prod_{j in G\T} Cf[f,j])
a bilinear form of rank 2^|G| -> computed on TensorE as fp32 matmuls with
PSUM accumulation (contraction over 2*2^|G| re/im-expanded rows).

v2 restructure (roles: E=C evicted-early, P=A PSUM-read, B=final-reduce):
- DMA over three channels so P(=A) completes early: Sync HWDGE carries
  C then B-chunk0; ACT HWDGE carries A-chunk1 then B-chunk1; GpSimd
  SWDGE carries A-chunk0.
- ACT evicts pgCre,pgCim,pgAim to fp16 and pgBre,pgBim scaled by 1/16
  (fp16 range); DVE runs the serial combine chain t1,t4 (PSUM 1x),
  t2,t3,U_,W_ (fp16 2x), then four fused multiply-reduce STTs against
  the evicted sBre/sBim (all-fp16 2x) -> out_t[:, 0..3].
- Host multiplies all four columns by 16 and assembles re/im.

DMA notes from the v1 sessions: both HWDGE rings (SP kick ~1.5us, ACT
kick ~2.3us to first packet, ~140 GB/s per ring), SWDGE ~2us completion
latency. Per-ring transfer receipts serialize (~+0.3-1us per later
transfer on the same ring).
"""

import numpy as np

N = 20
N_CORES = 8
F = 512           # free size (bits 0..8)
P = 128           # partitions (bits 9..15)
GA = list(range(0, 7))
GB = list(range(7, 14))
GC = list(range(14, 20))
W = 2 * P + F     # per-chunk packed width: [lhsT_re | lhsT_im | V]

_PROGRAM_CACHE = {}


def _signs(count, nbits):
    v = np.arange(count, dtype=np.int64)[:, None]
    return (((v >> np.arange(nbits)) & 1) * 2.0 - 1.0)  # (count, nbits) float64


def _subset_prods(C):
    """C: (nvals, g) complex128 -> (2^g, nvals); row T = prod_{k: bit k of T} C[:, k]."""
    out = np.ones((1, C.shape[0]), np.complex128)
    for k in range(C.shape[1]):
        out = np.concatenate([out, out * C[None, :, k]], axis=0)
    return out


def _pack_group(U, V):
    """Interleave re/im rows for the paired-contraction matmul layout.

    One shared V table streams through two matmuls; the re/im arithmetic is
    carried by two lhsT variants (contraction rows m = 2T + c):
      vtab[2T]   = Re V[T],  vtab[2T+1]   = Im V[T]
      lhs_re[2T] = Re U[T],  lhs_re[2T+1] = -Im U[T]   (-> PG_re)
      lhs_im[2T] = Im U[T],  lhs_im[2T+1] =  Re U[T]   (-> PG_im)
    """
    nT = U.shape[0]
    lre = np.empty((2 * nT, U.shape[1]), np.float32)
    lre[0::2] = U.real
    lre[1::2] = -U.imag
    lim = np.empty((2 * nT, U.shape[1]), np.float32)
    lim[0::2] = U.imag
    lim[1::2] = U.real
    vtab = np.empty((2 * nT, V.shape[1]), np.float32)
    vtab[0::2] = V.real
    vtab[1::2] = V.imag
    return lre, lim, vtab


def _build_core_tables(A, core):
    """Host tables for one core. A: (20,20) complex128.

    Each group packs to (128, nch*W): chunk k (contraction rows 128k..) at
    columns [k*W, (k+1)*W), laid out [lhsT_re | lhsT_im | V] per chunk.
    """
    f_signs = _signs(F, 9)
    p_signs = _signs(P, 7)
    c_signs = _signs(N_CORES, 3)
    par_f = np.prod(f_signs, axis=1)
    par_p = np.prod(p_signs, axis=1)
    par_c = np.prod(c_signs[core])

    Cf = f_signs @ A[0:9, :]                                         # (512, 20)
    Cp = p_signs @ A[9:16, :] + (c_signs[core] @ A[16:19, :] - A[19, :])[None, :]

    out = {}
    for name, G in (("A", GA), ("B", GB), ("C", GC)):
        U = _subset_prods(Cp[:, G])          # (2^g, 128)
        VV = _subset_prods(Cf[:, G])         # (2^g, 512)
        V = VV[::-1]                         # complement subset: T -> 2^g-1-T
        if name == "A":
            # fold full parity: par_p(p) * par_f(f) * par_c * (-1 for d19)
            U = U * (par_p[None, :] * (-par_c))
            V = V * par_f[None, :]
        lre, lim, vtab = _pack_group(U, V)
        packed = np.concatenate([lre, lim, vtab], axis=1).astype(np.float16)
        nch = packed.shape[0] // 128
        out["tab" + name] = np.ascontiguousarray(
            np.concatenate([packed[k * 128:(k + 1) * 128] for k in range(nch)],
                           axis=1))           # (128, nch*W)
    return out


def _build_program():
    if "prog" in _PROGRAM_CACHE:
        return _PROGRAM_CACHE["prog"]

    from contextlib import ExitStack
    from concourse import bass, mybir

    f32 = mybir.dt.float32
    f16 = mybir.dt.float16
    # FP16 tables: native 1-cycle/row PE path and half the DMA bytes of
    # fp32. Only the table values are rounded (2^-11); products accumulate
    # exactly in fp32 PSUM -> measured ~4e-4 end-to-end error.
    mm_dt = mybir.dt.float16
    mul = mybir.AluOpType.mult
    nc = bass.Bass()

    # DRAM parameters (per-core data via in_maps; same program on all cores).
    groups = (("A", 2), ("B", 2), ("C", 1))
    dram = {}
    for g, nch in groups:
        dram[g] = nc.declare_dram_parameter("tab" + g, [128, nch * W], mm_dt,
                                            isOutput=False)
    out_dram = nc.declare_dram_parameter("out", [P, 4], f32, isOutput=True)

    es = ExitStack()
    with es:
        # one semaphore per load DMA: sem increments come from the 16 SDMA
        # engines independently, so a shared semaphore with cumulative
        # thresholds would count a mix of both transfers' increments.
        dma_c = es.enter_context(nc.semaphore("dma_c"))
        dma_a0 = es.enter_context(nc.semaphore("dma_a0"))
        dma_a1 = es.enter_context(nc.semaphore("dma_a1"))
        dma_b0 = es.enter_context(nc.semaphore("dma_b0"))
        dma_b1 = es.enter_context(nc.semaphore("dma_b1"))
        pe_sem = es.enter_context(nc.semaphore("pe_sem"))
        act_sem = es.enter_context(nc.semaphore("act_sem"))
        dve_sem = es.enter_context(nc.semaphore("dve_sem"))

        sb = {}
        for g, nch in groups:
            sb[g] = es.enter_context(nc.sbuf_tensor("sb_tab" + g, [128, nch * W], mm_dt))
        names = ["sCre", "sCim", "sAim", "sBre", "sBim",
                 "t1", "t2", "t3", "t4", "U_", "W_", "scr"]
        wt = {n: es.enter_context(nc.sbuf_tensor(n, [P, F], f16)) for n in names}
        out_t = es.enter_context(nc.sbuf_tensor("out_t", [P, 4], f32))
        dummy = es.enter_context(nc.sbuf_tensor("actwarm", [P, 2], f32))
        pg = {}
        for g in ("A", "B", "C"):
            for comp in ("re", "im"):
                pg[g + comp] = es.enter_context(
                    nc.psum_tensor("pg" + g + comp, [P, F], f32))
        # NOTE: a PE "p-state warm-up" (dummy matmuls over uninitialized
        # scratch during the DMA wait) was measured +3us mean on every
        # core -- denormal/NaN garbage inputs appear to globally degrade
        # execution. Do not reintroduce it (zero-initialized scratch TBD).

        def _sync_stream():
            sync = nc.sync
            # Sync HWDGE ring: C (gates the first matmuls, smallest) then
            # B-chunk0 (needed ~5us later; receipt serialization is fine).
            sync.dma_start(sb["C"][:, :], dram["C"][:, :]).then_inc(dma_c, 16)
            sync.dma_start(sb["B"][:, :W], dram["B"][:, :W]).then_inc(dma_b0, 16)
            # out store: Sync is idle from here; wait for the 4 accumulator
            # reads (10 dve increments: 6 chain ops + 4 STT/read pairs).
            sync.wait_ge(dve_sem, 10)
            sync.dma_start(out_dram[:], out_t[:, :]).then_inc(dma_c, 16)

        def _act_stream():
            act = nc.scalar
            # ACT HWDGE ring: A-chunk1 then B-chunk1.
            act.dma_start(sb["A"][:, W:2 * W], dram["A"][:, W:2 * W]).then_inc(dma_a1, 16)
            act.dma_start(sb["B"][:, W:2 * W], dram["B"][:, W:2 * W]).then_inc(dma_b1, 16)
            # touch ACT before any gating wait so walrus's activation table
            # load happens during the DMA window, off the critical path
            # (reads uninitialized dummy SBUF -- the result is never used)
            act.copy(dummy[:, 1:2], dummy[:, 0:1])
            # fp16 evictions: C's outputs (role E) as each matmul retires,
            # then pgAim (t2/t3's operand), then B scaled by 1/16 (the
            # U_*PB / W_*PB products overflow fp16 unscaled; host undoes).
            act.wait_ge(pe_sem, 1)
            act.copy(wt["sCre"][:, :], pg["Cre"][:, :]).then_inc(act_sem, 1)
            act.wait_ge(pe_sem, 2)
            act.copy(wt["sCim"][:, :], pg["Cim"][:, :]).then_inc(act_sem, 1)
            act.wait_ge(pe_sem, 6)
            act.copy(wt["sAim"][:, :], pg["Aim"][:, :]).then_inc(act_sem, 1)
            act.wait_ge(pe_sem, 8)
            act.mul(wt["sBre"][:, :], pg["Bre"][:, :], 0.0625).then_inc(act_sem, 1)
            act.wait_ge(pe_sem, 10)
            act.mul(wt["sBim"][:, :], pg["Bim"][:, :], 0.0625).then_inc(act_sem, 1)

        def _pe_stream():
            pe = nc.tensor
            # matmul order C (2), A (4: re0,re1,im0,im1 so pgAre completes
            # earliest), B (4). pe_sem: pgC done at 2, pgAre 4, pgAim 6,
            # pgBre 8, pgBim 10.
            def mm(g, comp, k, pos, nch, wait=None, thr=0):
                if wait is not None:
                    pe.wait_ge(wait, thr)
                lo = k * W + (0 if comp == "re" else P)
                pe.matmul(
                    pg[g + comp][:, :],
                    sb[g][:, lo:lo + P],
                    sb[g][:, k * W + 2 * P:k * W + 2 * P + F],
                    start=(pos == 0),
                    stop=(pos == nch - 1),
                ).then_inc(pe_sem, 1)
            mm("C", "re", 0, 0, 1, dma_c, 16)
            mm("C", "im", 0, 0, 1)
            mm("A", "re", 0, 0, 2, dma_a0, 16)
            mm("A", "re", 1, 1, 2, dma_a1, 16)
            mm("A", "im", 0, 0, 2)
            mm("A", "im", 1, 1, 2)
            mm("B", "re", 0, 0, 2, dma_b0, 16)
            mm("B", "re", 1, 1, 2, dma_b1, 16)
            mm("B", "im", 0, 0, 2)
            mm("B", "im", 1, 1, 2)

        def _gpsimd_stream():
            gp = nc.gpsimd
            # SWDGE: A-chunk0 (~2us completion latency, hidden: first A
            # matmul isn't reachable before ~10us).
            gp.dma_start(sb["A"][:, :W], dram["A"][:, :W]).then_inc(dma_a0, 16)

        def _dve_stream():
            v = nc.vector
            # serial combine chain; engine order is program order so no
            # same-engine self-waits needed.
            # t1/t4 read pgAre straight from PSUM (1x): one eviction would
            # cost 687ns on ACT + delay; two 1x reads amortize better.
            v.wait_ge(act_sem, 1)
            v.wait_ge(pe_sem, 4)
            v.tensor_mul(wt["t1"][:, :], wt["sCre"][:, :], pg["Are"][:, :]).then_inc(dve_sem, 1)
            v.wait_ge(act_sem, 2)
            v.tensor_mul(wt["t4"][:, :], wt["sCim"][:, :], pg["Are"][:, :]).then_inc(dve_sem, 1)
            v.wait_ge(act_sem, 3)
            v.tensor_mul(wt["t2"][:, :], wt["sCim"][:, :], wt["sAim"][:, :]).then_inc(dve_sem, 1)
            v.tensor_mul(wt["t3"][:, :], wt["sCre"][:, :], wt["sAim"][:, :]).then_inc(dve_sem, 1)
            v.tensor_sub(wt["U_"][:, :], wt["t1"][:, :], wt["t2"][:, :]).then_inc(dve_sem, 1)
            v.tensor_add(wt["W_"][:, :], wt["t3"][:, :], wt["t4"][:, :]).then_inc(dve_sem, 1)
            # out cols: 0 = sum U*sBre, 1 = sum W*sBim, 2 = sum U*sBim,
            # 3 = sum W*sBre ; host computes re = 16(c0-c1), im = 16(c2+c3).
            # All-fp16 STTs (2x mode) against the ACT-evicted scaled B.
            v.wait_ge(act_sem, 4)
            v.scalar_tensor_tensor(
                wt["scr"][:, :], wt["U_"][:, :], 1.0, wt["sBre"][:, :],
                mul, mul, accum_out=out_t[:, 0:1]).then_inc(dve_sem, 1)
            v.scalar_tensor_tensor(
                wt["scr"][:, :], wt["W_"][:, :], 1.0, wt["sBre"][:, :],
                mul, mul, accum_out=out_t[:, 3:4]).then_inc(dve_sem, 1)
            v.wait_ge(act_sem, 5)
            v.scalar_tensor_tensor(
                wt["scr"][:, :], wt["U_"][:, :], 1.0, wt["sBim"][:, :],
                mul, mul, accum_out=out_t[:, 2:3]).then_inc(dve_sem, 1)
            v.scalar_tensor_tensor(
                wt["scr"][:, :], wt["W_"][:, :], 1.0, wt["sBim"][:, :],
                mul, mul, accum_out=out_t[:, 1:2]).then_inc(dve_sem, 1)

        _sync_stream()
        _act_stream()
        _pe_stream()
        _gpsimd_stream()
        _dve_stream()
        # no explicit epilogue: the NRT postamble quiesces DMA rings

    nc.finalize()
    _PROGRAM_CACHE["prog"] = nc
    return nc


def kernel(A_real, A_imag, _collect=None):
    from concourse.bass_utils import run_bass_kernel_spmd

    A = np.asarray(A_real, np.float64) + 1j * np.asarray(A_imag, np.float64)
    nc = _build_program()
    in_maps = [_build_core_tables(A, c) for c in range(N_CORES)]

    kwargs = dict(_collect or {})
    res = run_bass_kernel_spmd(nc, in_maps, core_ids=list(range(N_CORES)), **kwargs)
    if _collect is not None:
        _collect["results"] = res

    total = np.complex128(0)
    for r in res.results:
        o = np.asarray(r["out"], np.float64)
        # all four cols were computed against sB* = PB*/16 (fp16 range)
        total += 16.0 * ((o[:, 0] - o[:, 1]).sum() + 1j * (o[:, 2] + o[:, 3]).sum())

    perm = total * 2.0 * (2.0 ** (1 - N))
    ans = (perm.conjugate() * perm).real
    return np.asarray(ans, np.float32)
